# revision 7
# baseline (speedup 1.0000x reference)
"""Trainium2 Bass kernel for nn_DynamicMaxSimilarity — anti-diagonal rewrite.

Full inputs a,b: [512, 16, 256] f32.
  an = l2norm(tanh(a)) rows; bn likewise
  sim[a,b,i,j] = dot(an[a,i], bn[b,j]);  out[a,b] = DTW-like max-avg DP:
  si[i,j] = (max(si[i-1,j-1], si[i-1,j], si[i,j-1])*(m-1) + sim[i,j])/m,
  m = max(i,j), zero borders; answer si[16,16].

Sharding: 8 cores as 4 a-chunks (128) x 2 b-chunks (256). Per-core block
[128 a, 256 b]; pairs live as [128 partitions (a), 256 free (b)].

Design (vs the 201us L-border/scan baseline): process cells (i,j) by
anti-diagonal d=i+j in the *si domain*, which kills the per-slot
coefficient scaling and the 1.08ns/elem scan/STT ops entirely:
- state si kept as fp16 SBUF tiles U_d [128, 18*256] (phys slot = i,
  zero guard slots; 3 rotating buffers zero-initialized once).
- per diag: max1 = TT(U_{d-1}[i-1], U_{d-1}[i]); max2 = TT(max1,
  U_{d-2}[i-1]) — plain fp16 TTs run at 0.56 ns/elem (2x_1p), the only
  DVE work per cell.
- psum plane for (i,j) accumulates sim/m directly by pre-scaling the
  matmul operands (upper j>i: aT[i]*bTs[j], else aTs[i]*bT[j], where
  aTs = an/i, bTs = bn/j), then PE adds best*(m-1)/m via a diagonal
  weight matmul W=((m-1)/m)*I (contraction rows are free in PE cost;
  fp16 [*,256] matmul ~107-150ns). psum then holds si[i,j] exactly.
- ACT evicts whole-diag psum->U_d fp16 with no scale (batched copies).
- PSUM is one [128, 4096] tile = ring of 16 plane slots (cell n ->
  slot n mod 16); Tile subtile deps give the WAR ordering.
"""

import numpy as np

import concourse.bass as bass
from concourse import bacc
import concourse.mybir as mybir
from concourse.tile import TileContext
from concourse import bass_utils

NA, NB, T, D = 512, 512, 16, 256
ACH, BCH = 128, 256
P = 128
F = BCH              # psum cols per cell plane
KH = D // 128
DT = mybir.dt.float32
HT = mybir.dt.float16
IT = mybir.dt.int16
ALU = mybir.AluOpType
ACTF = mybir.ActivationFunctionType

_last_results = None


def _cells(d):
    i0, i1 = max(1, d - 16), min(16, d - 1)
    return list(range(i0, i1 + 1))


def _pieces(cells):
    """Split a diag's cell list into DVE/accum pieces: small leading
    pieces to cut the diag-to-diag latency chain, bigger tail pieces to
    amortize instruction overhead."""
    out = []
    i = 0
    sizes = [2, 2, 4, 4, 4]
    k = 0
    while i < len(cells):
        w = sizes[k] if k < len(sizes) else 8
        out.append(cells[i:i + w])
        i += w
        k += 1
    return out


def build_program():
    nc = bacc.Bacc("TRN2", target_bir_lowering=False, debug=False)

    a_d = nc.dram_tensor("a_c", [ACH, T, D], DT, kind="ExternalInput")
    b_d = nc.dram_tensor("b_c", [BCH, T, D], DT, kind="ExternalInput")
    out_d = nc.dram_tensor("out", [ACH, BCH], DT, kind="ExternalOutput")

    with TileContext(nc) as tc:
        with (
            tc.tile_pool(name="mp", bufs=1) as mp,
            tc.tile_pool(name="wp", bufs=2) as wp,
        ):
            # ---- load (fp32), quarters interleaved ----
            a_sb = mp.tile([P, T, D], DT, tag="ld_a")
            b_sb = [mp.tile([P, T, D], DT, name=f"b_sb{h}", tag=f"ld_b{h}")
                    for h in range(2)]
            for q in range(4):
                nc.sync.dma_start(a_sb[:, q * 4:(q + 1) * 4, :],
                                  a_d.ap()[:, q * 4:(q + 1) * 4, :])
            for h in range(2):
                for q in range(4):
                    nc.sync.dma_start(
                        b_sb[h][:, q * 4:(q + 1) * 4, :],
                        b_d.ap()[h * 128:(h + 1) * 128, q * 4:(q + 1) * 4, :])

            # ---- diagonal weight tiles W[m] = ((m-1)/m) * I_128, fp16 ----
            iota_t = mp.tile([P, 128], IT)
            nc.gpsimd.iota(iota_t[:, :], pattern=[[1, 128]], base=0,
                           channel_multiplier=-1)
            ident = mp.tile([P, 128], HT)
            nc.vector.tensor_scalar(ident[:, :], iota_t[:, :], 0, None,
                                    ALU.is_equal)
            wm = mp.tile([P, 16, 128], HT)
            for m in range(2, 17):
                nc.vector.tensor_scalar(wm[:, m - 1, :], ident[:, :],
                                        float((m - 1) / m), None, ALU.mult)

            # ---- DP state: si diag buffers, 18 slots (idx 0/17 guards) ----
            U = [mp.tile([P, 18 * F], HT, name=f"U{x}") for x in range(3)]
            for x in range(3):
                nc.gpsimd.memset(U[x][:, :], 0.0)
            B = [mp.tile([P, 18 * F], HT, name=f"B{x}") for x in range(2)]

            # ---- tanh -> fp16; sumsq; rinv; scales; transposes x2 ----
            ah = mp.tile([P, T, D], HT)
            bh = [mp.tile([P, T, D], HT, name=f"bh{h}") for h in range(2)]
            ssq = mp.tile([P, 3, T], DT)
            nrm = mp.tile([P, 3, T], DT)
            rinv = mp.tile([P, 3, T], DT)
            blocks = [(a_sb, ah, 0), (b_sb[0], bh[0], 1), (b_sb[1], bh[1], 2)]
            aT = mp.tile([P, T * KH, P], HT)        # [d, i*2+kh, a]
            aTs = mp.tile([P, T * KH, P], HT)       # scaled by 1/i
            bT = mp.tile([P, T, KH, 2, P], HT)      # [d, j, kh, half, b]
            bTs = mp.tile([P, T, KH, 2, P], HT)     # scaled by 1/j
            for x_sb, xh, bi in blocks:
                for hb in range(2):
                    for q in range(2 * hb, 2 * hb + 2):
                        sl = slice(q * 4, (q + 1) * 4)
                        nc.scalar.activation(xh[:, sl, :], x_sb[:, sl, :],
                                             ACTF.Tanh)
                        # sumsq: frame 4q on ACT (Square+accum), rest on DVE
                        sqa = wp.tile([P, D], HT, name=f"sqa{bi}_{q}",
                                      tag="sqa")
                        nc.scalar.activation(
                            sqa[:, :], xh[:, q * 4, :], ACTF.Square,
                            accum_out=ssq[:, bi, q * 4:q * 4 + 1])
                        sq = wp.tile([P, 3, D], HT, name=f"sq{bi}_{q}",
                                     tag="sq")
                        sl3 = slice(q * 4 + 1, (q + 1) * 4)
                        nc.vector.tensor_tensor(sq[:, :, :], xh[:, sl3, :],
                                                xh[:, sl3, :], ALU.mult)
                        nc.vector.tensor_reduce(ssq[:, bi, sl3], sq[:, :, :],
                                                mybir.AxisListType.X, ALU.add)
                    # rinv = rsqrt(ssq) via int bit trick + 1 Newton step
                    hs = slice(8 * hb, 8 * hb + 8)
                    sv = ssq[:, bi, hs]
                    yv = rinv[:, bi, hs]
                    wv = nrm[:, bi, hs]
                    nc.vector.tensor_scalar(yv.bitcast(mybir.dt.int32),
                                            sv.bitcast(mybir.dt.int32),
                                            1, None, ALU.logical_shift_right)
                    nc.vector.tensor_scalar(yv.bitcast(mybir.dt.int32),
                                            yv.bitcast(mybir.dt.int32),
                                            0x5F3759DF, -1,
                                            ALU.subtract, ALU.mult)
                    nc.vector.tensor_tensor(wv, yv, yv, ALU.mult)
                    nc.vector.tensor_tensor(wv, wv, sv, ALU.mult)
                    nc.vector.tensor_scalar(wv, wv, -0.5, 1.5,
                                            ALU.mult, ALU.add)
                    nc.vector.tensor_tensor(yv, yv, wv, ALU.mult)
                    for q in range(2 * hb, 2 * hb + 2):
                        for i in range(q * 4, (q + 1) * 4):
                            nc.vector.tensor_scalar_mul(
                                xh[:, i, :], xh[:, i, :],
                                rinv[:, bi, i:i + 1])
                        sl = slice(q * 4, (q + 1) * 4)
                        # scaled copy: xs[frame f] = xh[f] / (f+1)
                        xs = wp.tile([P, 4, D], HT, name=f"xs{bi}_{q}",
                                     tag="xs")
                        for i in range(q * 4, (q + 1) * 4):
                            nc.vector.tensor_scalar(
                                xs[:, i - q * 4, :], xh[:, i, :],
                                float(1.0 / (i + 1)), None, ALU.mult)
                        if bi == 0:
                            nc.sync.dma_start_transpose(
                                aT[:, q * 8:(q + 1) * 8, :], xh[:, sl, :])
                            nc.sync.dma_start_transpose(
                                aTs[:, q * 8:(q + 1) * 8, :], xs[:, :, :])
                        else:
                            nc.sync.dma_start_transpose(
                                bT[:, sl, :, bi - 1, :], xh[:, sl, :])
                            nc.sync.dma_start_transpose(
                                bTs[:, sl, :, bi - 1, :], xs[:, :, :])

            def amat(i, kh, scaled):
                # frame i is 1-based
                t = aTs if scaled else aT
                return t[:, (i - 1) * KH + kh, :]

            def bmov(j, kh, scaled):
                t = bTs if scaled else bT
                return t[:, j - 1, kh, :, :]

            # ---- DP over anti-diagonals ----
            # PSUM accumulation groups are PER BANK (a start=True matmul
            # into the other half of a bank kills the open group), so each
            # cell plane gets a full 2KB bank: ring of 8 banks, cell n ->
            # bank n mod 8 (first 256 of 512 cols used). A bank's chain
            # [sim kh0 (start), sim kh1, accum (stop)] for cell n must
            # fully precede cell n+8's chain in PE program order; sims are
            # therefore pumped in global cell order, gated on the eviction
            # of cell n-8 having been issued.
            with tc.tile_pool(name="pp", bufs=1, space="PSUM") as pp:
                PS = pp.tile([P, 16 * F], DT)   # 8 banks x 512 fp32

                n_of = {}
                cnt = 0
                order = []
                for dd in range(2, 33):
                    for ii in _cells(dd):
                        n_of[(dd, ii)] = cnt
                        order.append((dd, ii))
                        cnt += 1

                def bank(d, i):
                    return n_of[(d, i)] % 8

                def pcol(d, i):
                    return bank(d, i) * 2 * F

                state = {"sim": 0, "evicted": 0}

                def pump_sims():
                    # issue sim matmuls for every cell whose bank is free
                    while state["sim"] < 256 and \
                            state["sim"] < state["evicted"] + 8:
                        d, i = order[state["sim"]]
                        state["sim"] += 1
                        j = d - i
                        m = max(i, j)
                        c0 = pcol(d, i)
                        dst = PS[:, c0:c0 + F]
                        for kh in range(KH):
                            nc.tensor.matmul(
                                dst, amat(i, kh, scaled=(i >= j)),
                                bmov(j, kh, scaled=(j > i)),
                                start=(kh == 0),
                                stop=(kh == KH - 1 and m == 1))

                pump_sims()
                out_sb = mp.tile([P, F], DT)

                def useg(t, a, b):
                    return t[:, a * F:(b + 1) * F]

                for d in range(2, 33):
                    cells = _cells(d)
                    C = len(cells)
                    Ud = U[d % 3]
                    U1 = U[(d - 1) % 3]
                    U2 = U[(d - 2) % 3]
                    Bd = B[d % 2]
                    pieces = _pieces(cells)

                    # all maxes up front (DVE streams independently)
                    if d > 2:
                        for pc in pieces:
                            p0, p1 = pc[0], pc[-1]
                            nc.vector.tensor_tensor(
                                useg(Bd, p0, p1), useg(U1, p0 - 1, p1 - 1),
                                useg(U1, p0, p1), ALU.max)
                            nc.vector.tensor_tensor(
                                useg(Bd, p0, p1), useg(Bd, p0, p1),
                                useg(U2, p0 - 1, p1 - 1), ALU.max)

                    if d == 32:
                        i = 16
                        nc.tensor.matmul(
                            PS[:, pcol(32, 16):pcol(32, 16) + F],
                            wm[:, 15, :], Bd[:, i * F:(i + 1) * F],
                            start=False, stop=True)
                        nc.scalar.activation(
                            out_sb[:, :],
                            PS[:, pcol(32, 16):pcol(32, 16) + F], ACTF.Copy)
                        break

                    # per piece: accums -> evict runs -> pump freed sims
                    for pc in pieces:
                        if d > 2:
                            for i in pc:
                                m = max(i, d - i)
                                c0 = pcol(d, i)
                                nc.tensor.matmul(
                                    PS[:, c0:c0 + F], wm[:, m - 1, :],
                                    Bd[:, i * F:(i + 1) * F],
                                    start=False, stop=True)
                        # evict piece (split at bank ring wrap): psum
                        # banks are strided [2F]-wide, planes 256 valid
                        ri = pc[0]
                        rem = len(pc)
                        while rem > 0:
                            s = bank(d, ri)
                            w = min(rem, 8 - s)
                            src = PS[:, s * 2 * F:(s + w) * 2 * F].rearrange(
                                "p (c k) -> p c k", k=2 * F)[:, :, 0:F]
                            nc.scalar.activation(
                                useg(Ud, ri, ri + w - 1).rearrange(
                                    "p (c k) -> p c k", k=F),
                                src, ACTF.Copy)
                            ri += w
                            rem -= w
                        state["evicted"] += len(pc)
                        pump_sims()

            nc.sync.dma_start(out_d.ap(), out_sb[:, :])

    nc.compile()
    return nc


def kernel(a: np.ndarray, b: np.ndarray) -> np.ndarray:
    a = np.ascontiguousarray(a, dtype=np.float32)
    b = np.ascontiguousarray(b, dtype=np.float32)
    assert a.shape == (NA, T, D) and b.shape == (NB, T, D)

    nc = build_program()

    in_maps = []
    for core in range(8):
        ca, cb = core // 2, core % 2
        in_maps.append({
            "a_c": a[ca * ACH:(ca + 1) * ACH],
            "b_c": b[cb * BCH:(cb + 1) * BCH],
        })

    res = bass_utils.run_bass_kernel_spmd(nc, in_maps, core_ids=list(range(8)))
    global _last_results
    _last_results = res

    out = np.zeros((NA, NB), dtype=np.float32)
    for core in range(8):
        ca, cb = core // 2, core % 2
        out[ca * ACH:(ca + 1) * ACH, cb * BCH:(cb + 1) * BCH] = \
            res.results[core]["out"]
    return out
